# revision 16
# baseline (speedup 1.0000x reference)
"""Trainium2 Bass kernel for nn_CrossAttention (self-attention, B=2, S=2048,
16 heads x 64 dim, d_model=1024).

Sharding: batch*heads across 8 cores -> each core owns 2 heads for both
batches. Each core receives the full (pre-transposed, fp16) hidden states and
its 128-column slice of w_q/w_k/w_v (w_q pre-scaled by 1/sqrt(64)) plus its
128-row slice of w_o. Cores emit fp16 partial outputs [4096, 1024] (the w_o
contraction over the core's 128 inner dims); the host sums the partials in
fp32 and adds b_o.

Per-core dataflow (all matmuls fp16 operands, fp32 PSUM accumulation):
  1. Q^T/K^T [128, 4096] = w.T @ hs^T   (feature-major projections)
  2. V computed directly in [seq, feat] layout: per 128-seq tile,
     psum[seq,128] = hst_tile^T @ w_v (8 K-accumulation matmuls), copied
     into v65 with a ones column appended (softmax denominator rides the
     ctx matmul for free). No PE transposes needed.
  3. per (batch, q-chunk): scores^T[k, q] for BOTH heads packed into one
     PSUM tile via tile_position row groups (the two K=64 matmuls run
     concurrently on the PE array), one exp per packed tile on ACT,
     ctx^T[d, q] += [v | 1].T @ p^T per head
  4. normalize: DVE reciprocal -> gpsimd partition_broadcast -> DVE multiply
  5. out[q, 1024] = ctx^T.T @ w_o, interleaved into the attention blocks

Software pipelining (v3): ACT (exp) is the ~147us/iter roofline engine; it
must never starve. Projection bursts between attention blocks used to idle
ACT ~5us per block boundary (PE busy on projections, no fresh scores). Now
ALL projection work is fed into the attention kt-loop as small granules
(one proj-half or 4 direct-V tiles per ~5 kt steps), sized to fit in the
PE slack under each kt's exp. Groups 2,3 (batch 1) project during batch-0
attention of the same iteration; groups 0,1 project during batch-1
attention FOR THE NEXT ITERATION (the loop body reads q/k/v written by the
previous body; a pre-loop prologue projects groups 0,1 once). Out tiles
28-31 of iteration i store at the head of iteration i+1, with a post-loop
epilogue storing the final ones. The For_i uses staggered_reset (no
per-iteration all-engine barrier), so the tail attention of iteration i
runs straight into iteration i+1's head attention with no PE/ACT gap.

PSUM bank budget (8 banks of 2KB):
  tag A (scores, [128,1024] f32 = 2 banks) x2 bufs          -> 4 banks
  tag C (ctx accumulators, [65,512] f32 = 1 bank) x2 bufs   -> 2 banks
  tag B (proj/out transients, 1 bank) x1 buf                -> 1 bank
  tag F (keep-alive filler target, never read) x1 buf       -> 1 bank
"""
import numpy as np

HEADS = 16
DIM_HEAD = 64
QUERY_DIM = 1024
SCALE = DIM_HEAD ** -0.5
B, S = 2, 2048
NSEQ = B * S              # 4096
N_CORES = 8
FEAT = 128                # 2 heads x 64 per core
KT = S // 128             # 16 k-tiles per batch
N_FILL = 8                # keep-alive filler matmuls per kt step (HAM warmth)
DIAG_NO_ACT = False       # diagnostic: skip exp, ctx reads a constant tile

_nc_cache = {}


def _build(n_iters=1, unroll=1):
    import concourse.bass as bass
    import concourse.tile as tile
    from concourse import bacc, mybir
    from concourse.masks import make_identity

    F32 = mybir.dt.float32
    F16 = mybir.dt.float16
    AF = mybir.ActivationFunctionType

    nc = bacc.Bacc("TRN2", target_bir_lowering=False)

    hst = nc.dram_tensor("hst", [QUERY_DIM, NSEQ], F16, kind="ExternalInput")
    wq = nc.dram_tensor("wq", [QUERY_DIM, FEAT], F16, kind="ExternalInput")
    wk = nc.dram_tensor("wk", [QUERY_DIM, FEAT], F16, kind="ExternalInput")
    wv = nc.dram_tensor("wv", [QUERY_DIM, FEAT], F16, kind="ExternalInput")
    wo = nc.dram_tensor("wo", [FEAT, QUERY_DIM], F16, kind="ExternalInput")
    out = nc.dram_tensor("out", [NSEQ, QUERY_DIM], F16, kind="ExternalOutput")

    with tile.TileContext(nc) as tc:
        with (
            tc.tile_pool(name="sb", bufs=1) as sb,
            tc.tile_pool(name="ps", bufs=1, space="PSUM") as ps,
            tc.tile_pool(name="dr", bufs=1, space="DRAM") as dr,
        ):
            wq_sb = sb.tile([128, 8, FEAT], F16)
            wk_sb = sb.tile([128, 8, FEAT], F16)
            wv_sb = sb.tile([128, 8, FEAT], F16)
            wo_sb = sb.tile([128, QUERY_DIM], F16)

            qT_sb = sb.tile([128, NSEQ], F16)
            kT_sb = sb.tile([128, NSEQ], F16)
            ctxn_sb = sb.tile([128, NSEQ], F16)
            v65 = sb.tile([128, 4 * KT, 65], F16)
            # one-time setup (outside the loop): ones column for the softmax
            # denominator, ctxn zeroed (iteration 0's head reads the [1,3]
            # chunk before it's ever written), weight loads, and a dummy exp
            # to load the ACT Exp table. With Exp live on both loop edges,
            # insert_act_table_loads skips the per-iteration reload.
            nc.vector.memset(v65[:], 0.0)
            nc.vector.memset(v65[:, :, 64:65], 1.0)
            nc.vector.memset(ctxn_sb[:], 0.0)
            warm_exp = sb.tile([1, 64], F16)
            nc.scalar.activation(warm_exp[:], v65[0:1, 0, 0:64], AF.Exp)
            # keep-alive fillers: the PE HAM clock gate re-throttles to
            # 1.2 GHz when its 3.4us activity window sees the PE idle; the
            # per-kt PE->ACT ping-pong leaves micro-idles every ~1us, so the
            # PE can sit at half clock through the whole steady state. Tiny
            # dependency-free N=64 matmuls into a dedicated PSUM bank keep
            # the activity monitor fed. fill_sb is constant; p_fill is never
            # read, and WAW between fillers is same-engine program order (no
            # semaphores).
            ident = sb.tile([128, 128], F16)
            make_identity(nc, ident[:])
            pT_const = sb.tile([128, 1024], F16)
            nc.vector.memset(pT_const[:], 0.00048828125)  # 1/2048
            fill_sb = sb.tile([64, 64], F16)
            nc.vector.memset(fill_sb[:], 0.0)
            p_fill = ps.tile([1, 64], F32, tag="F", bufs=1, name="p_fill")

            def fillers(n):
                for _ in range(n):
                    nc.tensor.matmul(p_fill[:], fill_sb[:, 0:1],
                                     fill_sb[:, 0:64], start=True, stop=True)
            nc.scalar.dma_start(wk_sb[:], wk.ap().rearrange("(kt p) m -> p kt m", p=128))
            nc.scalar.dma_start(wq_sb[:], wq.ap().rearrange("(kt p) m -> p kt m", p=128))
            nc.scalar.dma_start(wv_sb[:], wv.ap().rearrange("(kt p) m -> p kt m", p=128))
            nc.scalar.dma_start(wo_sb[:], wo[:])

            hst_src = hst.ap().rearrange("(kt p) n -> p kt n", p=128)
            w_sbs = {"q": wq_sb, "k": wk_sb}
            v65_4d = v65.rearrange("p (pr kt) c -> p pr kt c", pr=4)

            def proj_half(proj, g, half, hst_t):
                """One 512-col half of the q or k projection for seq-group g."""
                g0 = g * 1024
                h0 = half * 512
                p_p = ps.tile([128, 512], F32, tag="B", bufs=1,
                              name=f"p_{proj}{g}_{half}")
                for kt in range(8):
                    nc.tensor.matmul(
                        p_p[:], w_sbs[proj][:, kt, :],
                        hst_t[:, kt, h0:h0 + 512],
                        start=(kt == 0), stop=(kt == 7),
                    )
                dst = qT_sb if proj == "q" else kT_sb
                nc.vector.tensor_copy(dst[:, g0 + h0:g0 + h0 + 512], p_p[:])

            def v_half(g, half, hst_t):
                """V^T projection for one 512-col half of group g (8 wide
                N=512 matmuls — fewer PE instructions than per-seq-tile
                direct-V), then 4 PE transposes fill v65's [seq, dim] slots.
                One strided copy per transpose covers both heads."""
                vT_t = sb.tile([128, 512], F16, tag="vT_t", bufs=2,
                               name=f"vT_t{g}_{half}")
                p_p = ps.tile([128, 512], F32, tag="B", bufs=1,
                              name=f"p_v{g}_{half}")
                h0 = half * 512
                for kt in range(8):
                    nc.tensor.matmul(
                        p_p[:], wv_sb[:, kt, :], hst_t[:, kt, h0:h0 + 512],
                        start=(kt == 0), stop=(kt == 7),
                    )
                nc.vector.tensor_copy(vT_t[:], p_p[:])
                for c in range(4):
                    ci = g * 8 + half * 4 + c
                    b_i, kt_loc = ci // 16, ci % 16
                    p_tr = ps.tile([128, 128], F16, tag="B", bufs=1,
                                   name=f"p_tr{ci}")
                    nc.tensor.transpose(
                        p_tr[:], vT_t[:, c * 128:(c + 1) * 128], ident[:])
                    nc.vector.tensor_copy(
                        v65_4d[:, b_i * 2:(b_i + 1) * 2, kt_loc, 0:64],
                        p_tr.rearrange("p (h d) -> p h d", h=2))

            def load_group(g, tag_name):
                hst_t = sb.tile([128, 8, 1024], F16, tag="hst_t", bufs=4,
                                name=tag_name)
                nc.sync.dma_start(hst_t[:], hst_src[:, :, g * 1024:(g + 1) * 1024])
                return hst_t

            def out_qt(qt, queue="sync"):
                """One 128-row tile of the final projection."""
                t0 = qt * 128
                o_sb = sb.tile([128, 1024], F16, tag="o_sb", bufs=6,
                               name=f"o_sb{qt}")
                for c in range(2):
                    p_o = ps.tile([128, 512], F32, tag="B", bufs=1,
                                  name=f"p_o{qt}_{c}")
                    nc.tensor.matmul(
                        p_o[:], ctxn_sb[:, t0:t0 + 128],
                        wo_sb[:, c * 512:(c + 1) * 512],
                        start=True, stop=True)
                    nc.vector.tensor_copy(o_sb[:, c * 512:(c + 1) * 512], p_o[:])
                eng = nc.scalar if queue == "scalar" else nc.sync
                eng.dma_start(out[t0:t0 + 128, :], o_sb[:])

            def attn_part(b_i, cc, kts, p_ctx, out_qts=(), feed=()):
                """Score+exp+ctx for kt in kts, (batch, 512-q-chunk cc),
                head-packed scores; out_qts and proj-granule closures (feed)
                interleaved into the PE slack under the exps."""
                s0 = b_i * S
                q0 = s0 + cc * 512
                oq = list(out_qts)
                fq = list(feed)
                kts = list(kts)
                # out tiles read ctxn of the previous chunk, whose normalize
                # chain completes a few us into this block -> second half.
                pop_at = {}
                if oq:
                    lo = len(kts) // 2
                    navail = max(len(kts) - lo, 1)
                    step = max(navail // len(oq), 1)
                    for i in range(len(oq)):
                        k = kts[min(lo + i * step, len(kts) - 1)]
                        pop_at.setdefault(k, []).append("o")
                if fq:
                    step = max(len(kts) // len(fq), 1)
                    for i in range(len(fq)):
                        k = kts[min(1 + i * step, len(kts) - 1)]
                        pop_at.setdefault(k, []).append("f")
                for kt in kts:
                    k0 = s0 + kt * 128
                    p_s = ps.tile([128, 1024], F32, tag="A", bufs=2,
                                  name=f"p_s{b_i}_{cc}_{kt}")
                    # head-packed: head h uses PE row group h*64, writes its
                    # own PSUM bank within the shared tile
                    for h in range(2):
                        hp = slice(h * 64, (h + 1) * 64)
                        nc.tensor.matmul(
                            p_s[:, h * 512:(h + 1) * 512],
                            kT_sb[hp, k0:k0 + 128],
                            qT_sb[hp, q0:q0 + 512],
                            start=True, stop=True,
                            tile_position=(h * 64, 0),
                        )
                    if DIAG_NO_ACT:
                        pT = pT_const
                    else:
                        pT = sb.tile([128, 1024], F16, tag="pT", bufs=12,
                                     name=f"pT{b_i}_{cc}_{kt}")
                        nc.scalar.activation(pT[:], p_s[:], AF.Exp)
                    for h in range(2):
                        nc.tensor.matmul(
                            p_ctx[h][:],
                            v65[:, (b_i * 2 + h) * 16 + kt, :],
                            pT[:, h * 512:(h + 1) * 512],
                            start=(kt == 0), stop=(kt == KT - 1),
                        )
                    fillers(N_FILL)
                    for kind in pop_at.get(kt, ()):
                        if kind == "o" and oq:
                            out_qt(oq.pop(0))
                        elif kind == "f" and fq:
                            fq.pop(0)()
                while oq:
                    out_qt(oq.pop(0))
                while fq:
                    fq.pop(0)()

            def ctx_tiles(b_i, cc):
                return [ps.tile([65, 512], F32, tag="C", bufs=2,
                                name=f"p_ctx{b_i}_{cc}_{h}")
                        for h in range(2)]

            def norm(b_i, cc, p_ctx):
                s0 = b_i * S
                q0 = s0 + cc * 512
                for h in range(2):
                    hp = slice(h * 64, (h + 1) * 64)
                    pc = p_ctx[h]
                    # copy out of PSUM first: frees the C slot in ~1us so the
                    # next chunk's ctx accumulation never waits on the (slow,
                    # DMA-round-trip) normalize chain below
                    ctxs = sb.tile([65, 512], F32, tag="ctxs", bufs=4,
                                   name=f"ctxs{b_i}_{cc}_{h}")
                    nc.vector.tensor_copy(ctxs[:], pc[:])
                    recip = sb.tile([1, 512], F32, tag="recip", bufs=4,
                                    name=f"recip{b_i}_{cc}_{h}")
                    nc.vector.reciprocal(recip[:], ctxs[64:65, :])
                    rbc_sb = sb.tile([64, 512], F32, tag="rbc", bufs=6,
                                     name=f"rbc{b_i}_{cc}_{h}")
                    nc.gpsimd.partition_broadcast(rbc_sb[:], recip[:])
                    nc.vector.tensor_mul(
                        ctxn_sb[hp, q0:q0 + 512], ctxs[0:64, :], rbc_sb[:])

            def attn_block(b_i, cc, out_qts, feed=()):
                p_ctx = ctx_tiles(b_i, cc)
                attn_part(b_i, cc, range(KT), p_ctx, out_qts, feed)
                norm(b_i, cc, p_ctx)

            # ---- prologue (once, outside the loop): project groups 0,1 so
            # the loop body can start attention immediately. ----
            hst_g0 = load_group(0, "hst_p0")
            hst_g1 = load_group(1, "hst_p1")
            for g, t in ((0, hst_g0), (1, hst_g1)):
                for half in range(2):
                    proj_half("k", g, half, t)
                    proj_half("q", g, half, t)
                    v_half(g, half, t)

            # ---- loop body: 8 attention blocks; each feeds 3 projection
            # granules (for batch 1 of this iteration, then batch 0 of the
            # next) plus 4 out tiles into its kt-loop. ----
            def emit_body():
                g2 = load_group(2, "hst_g2")
                g3 = load_group(3, "hst_g3")
                attn_block(0, 0, [28, 29, 30, 31],
                           [lambda: proj_half("k", 2, 0, g2),
                            lambda: v_half(2, 0, g2),
                            lambda: proj_half("q", 2, 0, g2)])
                attn_block(0, 1, [0, 1, 2, 3],
                           [lambda: proj_half("k", 2, 1, g2),
                            lambda: v_half(2, 1, g2),
                            lambda: proj_half("q", 2, 1, g2)])
                attn_block(0, 2, [4, 5, 6, 7],
                           [lambda: proj_half("k", 3, 0, g3),
                            lambda: v_half(3, 0, g3),
                            lambda: proj_half("q", 3, 0, g3)])
                g0 = load_group(0, "hst_g0")
                attn_block(0, 3, [8, 9, 10, 11],
                           [lambda: proj_half("k", 3, 1, g3),
                            lambda: v_half(3, 1, g3),
                            lambda: proj_half("q", 3, 1, g3)])
                attn_block(1, 0, [12, 13, 14, 15],
                           [lambda: proj_half("k", 0, 0, g0),
                            lambda: v_half(0, 0, g0),
                            lambda: proj_half("q", 0, 0, g0)])
                g1 = load_group(1, "hst_g1")
                attn_block(1, 1, [16, 17, 18, 19],
                           [lambda: proj_half("k", 0, 1, g0),
                            lambda: v_half(0, 1, g0),
                            lambda: proj_half("q", 0, 1, g0)])
                attn_block(1, 2, [20, 21, 22, 23],
                           [lambda: proj_half("k", 1, 0, g1),
                            lambda: v_half(1, 0, g1),
                            lambda: proj_half("q", 1, 0, g1)])
                attn_block(1, 3, [24, 25, 26, 27],
                           [lambda: proj_half("k", 1, 1, g1),
                            lambda: v_half(1, 1, g1),
                            lambda: proj_half("q", 1, 1, g1)])

            if n_iters == 1:
                for _u in range(unroll):
                    emit_body()
            else:
                # staggered_reset: back-edge goes straight to the body (sem
                # resets in stage preambles) -> no per-iteration all-engine
                # barrier; iteration i's tail attention runs straight into
                # iteration i+1's head.
                with tc.For_i(0, n_iters, 1, staggered_reset=True):
                    for _u in range(unroll):
                        emit_body()

            # ---- epilogue: the last iteration's final out tiles (the body
            # defers them to the next iteration's head). ----
            for i, qt in enumerate(range(28, 32)):
                out_qt(qt, queue=("scalar" if i % 2 else "sync"))

    nc.finalize()
    return nc


def _get_nc(n_iters=1, unroll=1):
    key = (n_iters, unroll)
    if key not in _nc_cache:
        _nc_cache[key] = _build(n_iters, unroll)
    return _nc_cache[key]


def prepare_in_maps(hidden_states, w_q, w_k, w_v, w_o):
    hs = np.asarray(hidden_states, dtype=np.float32).reshape(NSEQ, QUERY_DIM)
    hst = np.ascontiguousarray(hs.T).astype(np.float16)
    wqs = (np.asarray(w_q, dtype=np.float32) * SCALE).astype(np.float16)
    wk16 = np.asarray(w_k, dtype=np.float32).astype(np.float16)
    wv16 = np.asarray(w_v, dtype=np.float32).astype(np.float16)
    wo16 = np.asarray(w_o, dtype=np.float32).astype(np.float16)
    in_maps = []
    for d in range(N_CORES):
        cols = slice(d * FEAT, (d + 1) * FEAT)
        in_maps.append({
            "hst": hst,
            "wq": np.ascontiguousarray(wqs[:, cols]),
            "wk": np.ascontiguousarray(wk16[:, cols]),
            "wv": np.ascontiguousarray(wv16[:, cols]),
            "wo": np.ascontiguousarray(wo16[cols, :]),
        })
    return in_maps


def run_spmd(in_maps, n_iters=1, unroll=1, **kwargs):
    from concourse.bass_utils import run_bass_kernel_spmd
    return run_bass_kernel_spmd(_get_nc(n_iters, unroll), in_maps,
                                core_ids=list(range(N_CORES)), **kwargs)


def kernel(hidden_states, w_q, w_k, w_v, w_o, b_o):
    in_maps = prepare_in_maps(hidden_states, w_q, w_k, w_v, w_o)
    res = run_spmd(in_maps)
    acc = np.zeros((NSEQ, QUERY_DIM), dtype=np.float32)
    for r in res.results:
        acc += r["out"].astype(np.float32)
    acc += np.asarray(b_o, dtype=np.float32)
    return acc.reshape(B, S, QUERY_DIM)
